# revision 1
# baseline (speedup 1.0000x reference)
"""Echo State Network kernel for Trainium2 (8 NeuronCores, batch-sharded).

Math (per reference):
    h_{t}   = tanh(W_in x_t + b_res + W_res h_{t-1}),  h in R^{2048}, T=1024
    y_t     = W_out h_t + b_out

Design:
  - Data-parallel over batch: 32 sequences -> 4 per core, replicated weights.
  - Fused contraction: W' = [W_res | W_in | b_res | 0] of shape [2048, 2176].
    Each step is one PE pass: out[n, b] = sum_k W'[n, k] * s[k, b] with
    s = [h; x_t; 1; 0]. Stationary operand = W'^T tiles (bf16, FWL), moving
    operand = state columns. Output lands reservoir-major [128, (chunk, b)]
    so no transpose is ever needed; ACT applies tanh and casts to bf16.
  - States stream to DRAM each step; a separate readout phase does
    y = W_out^T-contracted matmuls at N=512 efficiency.
"""

from contextlib import ExitStack

import numpy as np
import ml_dtypes

import concourse.bass as bass
import concourse.tile as tile
from concourse import bacc, mybir
from concourse.bass import ds
from concourse.bass_utils import run_bass_kernel_spmd

BF16 = mybir.dt.bfloat16
F32 = mybir.dt.float32
AF = mybir.ActivationFunctionType

N_CORES = 8
B, T_FULL, N_IN, N_RES, N_OUT = 32, 1024, 64, 2048, 64
BL = B // N_CORES          # 4 sequences per core
NCH = N_RES // 128         # 16 output chunks of 128
KCH = NCH + 1              # contraction chunks: 16 reservoir + 1 (x, bias)
KDIM = KCH * 128           # 2176 padded contraction size
UNROLL = 2                 # steps per For_i iteration
TB = 128                   # readout time-block (=> N=512 matmuls)

LAST_RESULTS = None        # BassKernelResults of the most recent run (for test.py)


def build_module(T=T_FULL, repeat=1):
    nc = bacc.Bacc("TRN2")
    wt = nc.dram_tensor("wt", [KDIM, N_RES], BF16, kind="ExternalInput")
    xb = nc.dram_tensor("xb", [128, T * BL], BF16, kind="ExternalInput")
    wo = nc.dram_tensor("wo", [N_RES, N_OUT], BF16, kind="ExternalInput")
    bo = nc.dram_tensor("bo", [N_OUT, 1], F32, kind="ExternalInput")
    states = nc.dram_tensor("states", [T, 128, NCH * BL], BF16)
    y = nc.dram_tensor("y", [N_OUT, T * BL], F32, kind="ExternalOutput")

    with tile.TileContext(nc) as tc, ExitStack() as ctx:
        singles = ctx.enter_context(tc.tile_pool(name="singles", bufs=1))
        psum_pool = ctx.enter_context(
            tc.tile_pool(name="psum", bufs=2, space="PSUM")
        )

        w_sb = singles.tile([128, KCH * N_RES], BF16)
        nc.sync.dma_start(
            w_sb[:].rearrange("p (j n) -> p j n", n=N_RES),
            wt.rearrange("(j p) n -> p j n", p=128),
        )
        xb_sb = singles.tile([128, T * BL], BF16)
        nc.sync.dma_start(xb_sb[:], xb[:, :])
        wo_sb = singles.tile([128, NCH * N_OUT], BF16)
        nc.sync.dma_start(
            wo_sb[:].rearrange("p (k o) -> p k o", o=N_OUT),
            wo.rearrange("(k p) o -> p k o", p=128),
        )
        bo_sb = singles.tile([N_OUT, 1], F32)
        nc.sync.dma_start(bo_sb[:], bo[:, :])

        # Ping-pong state tiles, reservoir-major: H[p, BL*j + b] = h[128j+p, b]
        H0 = singles.tile([128, NCH * BL], BF16)
        H1 = singles.tile([128, NCH * BL], BF16)
        nc.vector.memset(H0[:], 0.0)

        def w_tile(j, i):
            base = N_RES * j + 128 * i
            return w_sb[:, base : base + 128]

        def step(t_expr, Hsrc, Hdst):
            ps = psum_pool.tile([128, NCH * BL], F32, tag="ps")
            xcol = xb_sb[:, ds(t_expr * BL, BL)]
            for i in range(NCH):
                for j in range(KCH):
                    rhs = Hsrc[:, BL * j : BL * (j + 1)] if j < NCH else xcol
                    nc.tensor.matmul(
                        ps[:, BL * i : BL * (i + 1)],
                        w_tile(j, i),
                        rhs,
                        start=(j == 0),
                        stop=(j == KCH - 1),
                    )
            nc.scalar.activation(Hdst[:], ps[:], AF.Tanh)
            nc.sync.dma_start(
                states[ds(t_expr, 1)].rearrange("o p f -> (o p) f"), Hdst[:]
            )

        for _rep in range(repeat):
            if _rep > 0:
                nc.vector.memset(H0[:], 0.0)
            with tc.For_i(
                0, T, UNROLL, hint_engines=(mybir.EngineType.PE,)
            ) as iv:
                for s in range(UNROLL):
                    Hsrc, Hdst = (H0, H1) if s % 2 == 0 else (H1, H0)
                    step(iv + s, Hsrc, Hdst)

        # Readout: y[o, (t, b)] = sum_n W_out[o, n] h_t[n, b] + b_out[o]
        st_pool = ctx.enter_context(tc.tile_pool(name="st", bufs=2))
        ysb_pool = ctx.enter_context(tc.tile_pool(name="ysb", bufs=2))
        ypsum_pool = ctx.enter_context(
            tc.tile_pool(name="ypsum", bufs=2, space="PSUM")
        )
        TBv = min(TB, T)
        for tb in range(T // TBv):
            st = st_pool.tile([128, TBv * NCH * BL], BF16, tag="st")
            nc.sync.dma_start(
                st[:].rearrange("p (t f) -> p t f", f=NCH * BL),
                states[tb * TBv : (tb + 1) * TBv].rearrange("t p f -> p t f"),
            )
            st3 = st[:].rearrange("p (t f) -> p t f", f=NCH * BL)
            yp = ypsum_pool.tile([N_OUT, TBv * BL], F32, tag="yp")
            for k in range(NCH):
                nc.tensor.matmul(
                    yp[:],
                    wo_sb[:, N_OUT * k : N_OUT * (k + 1)],
                    st3[:, :, BL * k : BL * (k + 1)],
                    start=(k == 0),
                    stop=(k == NCH - 1),
                )
            ysb = ysb_pool.tile([N_OUT, TBv * BL], F32, tag="ysb")
            nc.vector.tensor_scalar_add(ysb[:], yp[:], bo_sb[:, 0:1])
            nc.sync.dma_start(y[:, tb * TBv * BL : (tb + 1) * TBv * BL], ysb[:])

    nc.finalize()
    return nc


def prep_inputs(x, W_in, W_res, b_res, W_out, b_out, T=T_FULL):
    bf = ml_dtypes.bfloat16
    Wp = np.concatenate(
        [
            W_res,
            W_in,
            b_res[:, None],
            np.zeros((N_RES, KDIM - N_RES - N_IN - 1), np.float32),
        ],
        axis=1,
    )
    wt = np.ascontiguousarray(Wp.T).astype(bf)            # [2176, 2048]
    wo = np.ascontiguousarray(W_out.T).astype(bf)         # [2048, 64]
    bo = np.ascontiguousarray(b_out[:, None]).astype(np.float32)
    in_maps = []
    for c in range(N_CORES):
        xs = x[BL * c : BL * (c + 1), :T]                 # [BL, T, N_IN]
        xbc = np.zeros((128, T * BL), bf)
        xbc[:N_IN] = (
            np.ascontiguousarray(xs.transpose(2, 1, 0).reshape(N_IN, T * BL))
            .astype(bf)
        )
        xbc[N_IN] = bf(1.0)
        in_maps.append({"wt": wt, "xb": xbc, "wo": wo, "bo": bo})
    return in_maps


def assemble_output(results, T=T_FULL):
    y = np.empty((B, T, N_OUT), np.float32)
    for c in range(N_CORES):
        yc = results[c]["y"]                              # [64, T*BL]
        y[BL * c : BL * (c + 1)] = (
            yc.reshape(N_OUT, T, BL).transpose(2, 1, 0)
        )
    return y


def run(x, W_in, W_res, b_res, W_out, b_out, T=T_FULL, **run_kwargs):
    global LAST_RESULTS
    in_maps = prep_inputs(x, W_in, W_res, b_res, W_out, b_out, T=T)
    nc = build_module(T=T)
    res = run_bass_kernel_spmd(
        nc, in_maps, core_ids=list(range(N_CORES)), **run_kwargs
    )
    LAST_RESULTS = res
    return assemble_output(res.results, T=T)


def kernel(x, W_in, W_res, b_res, W_out, b_out):
    return run(
        np.asarray(x, np.float32),
        np.asarray(W_in, np.float32),
        np.asarray(W_res, np.float32),
        np.asarray(b_res, np.float32),
        np.asarray(W_out, np.float32),
        np.asarray(b_out, np.float32),
    )



# revision 3
# speedup vs baseline: 1.0166x; 1.0166x over previous
"""Echo State Network kernel for Trainium2 — single-core, full batch.

Math (per reference):
    h_t = tanh(W_in x_t + b_res + W_res h_{t-1}),  h in R^2048, T=1024
    y_t = W_out h_t + b_out

Why one core: each step must stream all of W_res into the PE array
(stationary-operand loads dominate; the moving operand is the batch and
is nearly free up to ~64 columns). 32 sequences on one core cost the
same per step as 4, and single-device dispatch avoids the 8-way
shard_map overhead. The other cores idle.

Design:
  - Fused contraction: W' = [W_res | W_in | b_res | 0] of shape
    [2048, 2176], stored transposed as 17 k-chunks of [128, 2048] in
    SBUF. Each step: 16 output chunks x 17 contraction chunks of
    (LDWEIGHTS + matmul N=32), accumulating into 2 PSUM banks
    (chunks 0-7 / 8-15). ACT applies tanh per half so the next step's
    early k-chunks unblock while the late ones are still activating.
  - States live only in an SBUF ring of RING=8 steps; every RING steps
    the readout y = W_out h + b_out runs as 16 matmuls of N=256
    directly from the ring (states never touch DRAM).
  - For_i unrolled by RING so ring slots are static addresses.
"""

from contextlib import ExitStack

import numpy as np
import ml_dtypes

import concourse.bass as bass
import concourse.tile as tile
from concourse import bacc, mybir
from concourse.bass import ds
from concourse.bass_utils import run_bass_kernel_spmd

BF16 = mybir.dt.bfloat16
F32 = mybir.dt.float32
AF = mybir.ActivationFunctionType

N_CORES = 1
B, T_FULL, N_IN, N_RES, N_OUT = 32, 1024, 64, 2048, 64
NCH = N_RES // 128         # 16 output chunks of 128
KCH = NCH + 1              # contraction chunks: 16 reservoir + 1 (x, bias)
KDIM = KCH * 128           # 2176 padded contraction size
RING = 8                   # SBUF state-ring depth = steps per For_i iter
HB = NCH * B               # 512: one step's state row [128, HB]

LAST_RESULTS = None        # BassKernelResults of the most recent run (for test.py)


def build_module(T=T_FULL, repeat=1):
    nc = bacc.Bacc("TRN2")
    wt = nc.dram_tensor("wt", [KDIM, N_RES], BF16, kind="ExternalInput")
    xb = nc.dram_tensor("xb", [128, T * B], BF16, kind="ExternalInput")
    wo = nc.dram_tensor("wo", [N_RES, N_OUT], BF16, kind="ExternalInput")
    bo = nc.dram_tensor("bo", [N_OUT, 1], F32, kind="ExternalInput")
    y = nc.dram_tensor("y", [N_OUT, T * B], F32, kind="ExternalOutput")

    with tile.TileContext(nc) as tc, ExitStack() as ctx:
        singles = ctx.enter_context(tc.tile_pool(name="singles", bufs=1))
        psum_pool = ctx.enter_context(
            tc.tile_pool(name="psum", bufs=2, space="PSUM")
        )
        ypsum_pool = ctx.enter_context(
            tc.tile_pool(name="ypsum", bufs=2, space="PSUM")
        )
        ysb_pool = ctx.enter_context(tc.tile_pool(name="ysb", bufs=2))

        w_sb = singles.tile([128, KCH * N_RES], BF16)
        nc.sync.dma_start(
            w_sb[:].rearrange("p (j n) -> p j n", n=N_RES),
            wt.rearrange("(j p) n -> p j n", p=128),
        )
        xb_sb = singles.tile([128, T * B], BF16)
        nc.sync.dma_start(xb_sb[:], xb[:, :])
        wo_sb = singles.tile([128, NCH * N_OUT], BF16)
        nc.sync.dma_start(
            wo_sb[:].rearrange("p (k o) -> p k o", o=N_OUT),
            wo.rearrange("(k p) o -> p k o", p=128),
        )
        bo_sb = singles.tile([N_OUT, 1], F32)
        nc.sync.dma_start(bo_sb[:], bo[:, :])

        # State ring, reservoir-major: slot s, chunk j, batch b at
        # Hring[p, 512*s + 32*j + b] = h[128*j + p, b] of step t=s (mod RING).
        Hring = singles.tile([128, RING * HB], BF16)
        Hr3 = Hring[:].rearrange("p (s f) -> p s f", f=HB)

        def w_tile(j, i):
            base = N_RES * j + 128 * i
            return w_sb[:, base : base + 128]

        def step(t_expr, s):
            # MMs for output chunks 0-7 -> ps_a (one PSUM bank),
            # 8-15 -> ps_b, each group: x-chunk first, then k-chunks in
            # the order the previous step produced them.
            ps_a = psum_pool.tile([128, 8 * B], F32, tag="psa")
            ps_b = psum_pool.tile([128, 8 * B], F32, tag="psb")
            xcol = xb_sb[:, ds(t_expr * B, B)]
            src = Hring[:, (s - 1) % RING * HB : ((s - 1) % RING + 1) * HB]
            for i in range(NCH):
                ps = ps_a if i < 8 else ps_b
                out = ps[:, B * (i % 8) : B * (i % 8 + 1)]
                nc.tensor.matmul(
                    out, w_tile(KCH - 1, i), xcol, start=True, stop=False
                )
                for j in range(NCH):
                    nc.tensor.matmul(
                        out,
                        w_tile(j, i),
                        src[:, B * j : B * (j + 1)],
                        start=False,
                        stop=(j == NCH - 1),
                    )
            dst = Hring[:, s * HB : (s + 1) * HB]
            nc.scalar.activation(dst[:, : 8 * B], ps_a[:], AF.Tanh)
            nc.scalar.activation(dst[:, 8 * B :], ps_b[:], AF.Tanh)

        for _rep in range(repeat):
            # h_{-1} = 0 lives in slot RING-1 (read by step 0).
            nc.vector.memset(Hring[:, (RING - 1) * HB : RING * HB], 0.0)
            with tc.For_i(
                0, T, RING, hint_engines=(mybir.EngineType.PE,)
            ) as iv:
                for s in range(RING):
                    step(iv + s, s)
                # Readout for the RING steps just produced:
                # y[o, (t, b)] = sum_k W_out[o, 128k+p] h[128k+p, (t, b)]
                yp = ypsum_pool.tile([N_OUT, RING * B], F32, tag="yp")
                for k in range(NCH):
                    nc.tensor.matmul(
                        yp[:],
                        wo_sb[:, N_OUT * k : N_OUT * (k + 1)],
                        Hr3[:, :, B * k : B * (k + 1)],
                        start=(k == 0),
                        stop=(k == NCH - 1),
                    )
                ysb = ysb_pool.tile([N_OUT, RING * B], F32, tag="ysb")
                nc.vector.tensor_scalar_add(ysb[:], yp[:], bo_sb[:, 0:1])
                nc.sync.dma_start(y[:, ds(iv * B, RING * B)], ysb[:])

    nc.finalize()
    return nc


def prep_inputs(x, W_in, W_res, b_res, W_out, b_out, T=T_FULL):
    bf = ml_dtypes.bfloat16
    Wp = np.concatenate(
        [
            W_res,
            W_in,
            b_res[:, None],
            np.zeros((N_RES, KDIM - N_RES - N_IN - 1), np.float32),
        ],
        axis=1,
    )
    wt = np.ascontiguousarray(Wp.T).astype(bf)            # [2176, 2048]
    wo = np.ascontiguousarray(W_out.T).astype(bf)         # [2048, 64]
    bo = np.ascontiguousarray(b_out[:, None]).astype(np.float32)
    xs = x[:, :T]                                         # [B, T, N_IN]
    xbc = np.zeros((128, T * B), bf)
    xbc[:N_IN] = (
        np.ascontiguousarray(xs.transpose(2, 1, 0).reshape(N_IN, T * B))
        .astype(bf)
    )
    xbc[N_IN] = bf(1.0)
    return [{"wt": wt, "xb": xbc, "wo": wo, "bo": bo}]


def assemble_output(results, T=T_FULL):
    yc = results[0]["y"]                                  # [64, T*B]
    return np.ascontiguousarray(
        yc.reshape(N_OUT, T, B).transpose(2, 1, 0)
    )


def run(x, W_in, W_res, b_res, W_out, b_out, T=T_FULL, **run_kwargs):
    global LAST_RESULTS
    in_maps = prep_inputs(x, W_in, W_res, b_res, W_out, b_out, T=T)
    nc = build_module(T=T)
    res = run_bass_kernel_spmd(
        nc, in_maps, core_ids=list(range(N_CORES)), **run_kwargs
    )
    LAST_RESULTS = res
    return assemble_output(res.results, T=T)


def kernel(x, W_in, W_res, b_res, W_out, b_out):
    return run(
        np.asarray(x, np.float32),
        np.asarray(W_in, np.float32),
        np.asarray(W_res, np.float32),
        np.asarray(b_res, np.float32),
        np.asarray(W_out, np.float32),
        np.asarray(b_out, np.float32),
    )


# revision 12
# speedup vs baseline: 2.0915x; 2.0573x over previous
"""Echo State Network kernel for Trainium2 — single-core, full batch.

Math (per reference):
    h_t = tanh(W_in x_t + b_res + W_res h_{t-1}),  h in R^2048, T=1024
    y_t = W_out h_t + b_out

Why one core: each step must stream all of W_res into the PE array
(stationary-operand loads dominate; the moving operand is the batch and
is nearly free up to ~64 columns). 32 sequences on one core cost the
same per step as 4, and single-device dispatch avoids the 8-way
shard_map overhead. The other cores idle.

Design:
  - Fused contraction: W' = [W_res | W_in | b_res | 0] of shape
    [2048, 2176], stored transposed as 17 k-chunks of [128, 2048] in
    SBUF. Each step: 16 output chunks x 17 contraction chunks of
    (LDWEIGHTS + matmul N=32), accumulating into 2 PSUM banks
    (chunks 0-7 / 8-15). ACT applies tanh per half so the next step's
    early k-chunks unblock while the late ones are still activating.
  - Hybrid fp8: the first NF8 k-chunks of W_res are stored fp8-e4m3
    (stationary-side only; the moving state stays bf16 - mixed-dtype
    matmul is exact on HW). FWL loads fp8 weights 2x faster than bf16,
    and LDWEIGHTS bandwidth is the step bottleneck. All weights are
    pre-scaled by 32 (exact for bf16, lifts fp8 out of subnormals);
    the tanh activation's free scale=1/32 undoes it. Measured rel err
    vs the f32 reference: 0.0142 (NF8=8) vs 0.0032 all-bf16, tol 2e-2.
  - States live only in an SBUF ring of RING=8 steps; every RING steps
    the readout y = W_out h + b_out runs as 16 matmuls of N=256
    directly from the ring (states never touch DRAM).
  - For_i unrolled by RING so ring slots are static addresses.
"""

from contextlib import ExitStack

import numpy as np
import ml_dtypes

import concourse.bass as bass
import concourse.tile as tile
from concourse import bacc, mybir
from concourse.bass import ds
from concourse.bass_utils import run_bass_kernel_spmd

BF16 = mybir.dt.bfloat16
F8 = mybir.dt.float8e4
F32 = mybir.dt.float32
AF = mybir.ActivationFunctionType

N_CORES = 1
B, T_FULL, N_IN, N_RES, N_OUT = 32, 1024, 64, 2048, 64
NCH = N_RES // 128         # 16 output chunks of 128
KCH = NCH + 1              # contraction chunks: 16 reservoir + 1 (x, bias)
KDIM = KCH * 128           # 2176 padded contraction size
RING = 8                   # SBUF state-ring depth = steps per For_i iter
HB = NCH * B               # 512: one step's state row [128, HB]
NF8 = 8                    # k-chunks 0..NF8-1 of W_res stored in fp8
WSCALE = 32.0              # weight pre-scale (undone by tanh's scale=1/32)

LAST_RESULTS = None        # BassKernelResults of the most recent run (for test.py)


def build_module(T=T_FULL, repeat=1):
    nc = bacc.Bacc("TRN2")
    wt8 = nc.dram_tensor("wt8", [NF8 * 128, N_RES], F8, kind="ExternalInput")
    wtb = nc.dram_tensor(
        "wtb", [(KCH - NF8) * 128, N_RES], BF16, kind="ExternalInput"
    )
    xb = nc.dram_tensor("xb", [128, T * B], BF16, kind="ExternalInput")
    wo = nc.dram_tensor("wo", [N_RES, N_OUT], BF16, kind="ExternalInput")
    bo = nc.dram_tensor("bo", [N_OUT, 1], F32, kind="ExternalInput")
    y = nc.dram_tensor("y", [N_OUT, T * B], F32, kind="ExternalOutput")

    with tile.TileContext(nc) as tc, ExitStack() as ctx:
        singles = ctx.enter_context(tc.tile_pool(name="singles", bufs=1))
        psum_pool = ctx.enter_context(
            tc.tile_pool(name="psum", bufs=2, space="PSUM")
        )
        ypsum_pool = ctx.enter_context(
            tc.tile_pool(name="ypsum", bufs=2, space="PSUM")
        )
        ysb_pool = ctx.enter_context(tc.tile_pool(name="ysb", bufs=2))

        w8_sb = singles.tile([128, NF8 * N_RES], F8)
        nc.sync.dma_start(
            w8_sb[:].rearrange("p (j n) -> p j n", n=N_RES),
            wt8.rearrange("(j p) n -> p j n", p=128),
        )
        wb_sb = singles.tile([128, (KCH - NF8) * N_RES], BF16)
        nc.sync.dma_start(
            wb_sb[:].rearrange("p (j n) -> p j n", n=N_RES),
            wtb.rearrange("(j p) n -> p j n", p=128),
        )
        xb_sb = singles.tile([128, T * B], BF16)
        nc.sync.dma_start(xb_sb[:], xb[:, :])
        wo_sb = singles.tile([128, NCH * N_OUT], BF16)
        nc.sync.dma_start(
            wo_sb[:].rearrange("p (k o) -> p k o", o=N_OUT),
            wo.rearrange("(k p) o -> p k o", p=128),
        )
        bo_sb = singles.tile([N_OUT, 1], F32)
        nc.sync.dma_start(bo_sb[:], bo[:, :])

        # State ring, reservoir-major: slot s, chunk j, batch b at
        # Hring[p, 512*s + 32*j + b] = h[128*j + p, b] of step t=s (mod RING).
        Hring = singles.tile([128, RING * HB], BF16)
        Hr3 = Hring[:].rearrange("p (s f) -> p s f", f=HB)

        def w_tile(j, i):
            if j < NF8:
                base = N_RES * j + 128 * i
                return w8_sb[:, base : base + 128]
            base = N_RES * (j - NF8) + 128 * i
            return wb_sb[:, base : base + 128]

        def step(t_expr, s):
            # MMs for output chunks 0-7 -> ps_a (one PSUM bank),
            # 8-15 -> ps_b. Two phases: first every group's x-chunk and
            # early k-chunks (0..7, which the previous step's first tanh
            # produced), then every group's late k-chunks (8..15). This
            # gives the PE ~16x9 matmuls of ready work while the previous
            # step's second tanh half is still in flight.
            ps_a = psum_pool.tile([128, 8 * B], F32, tag="psa")
            ps_b = psum_pool.tile([128, 8 * B], F32, tag="psb")
            xcol = xb_sb[:, ds(t_expr * B, B)]
            src = Hring[:, (s - 1) % RING * HB : ((s - 1) % RING + 1) * HB]

            def out_ap(i):
                ps = ps_a if i < 8 else ps_b
                return ps[:, B * (i % 8) : B * (i % 8 + 1)]

            for i in range(NCH):
                out = out_ap(i)
                nc.tensor.matmul(
                    out, w_tile(KCH - 1, i), xcol, start=True, stop=False
                )
                for j in range(NCH // 2):
                    nc.tensor.matmul(
                        out,
                        w_tile(j, i),
                        src[:, B * j : B * (j + 1)],
                        start=False,
                        stop=False,
                    )
            for i in range(NCH):
                out = out_ap(i)
                for j in range(NCH // 2, NCH):
                    nc.tensor.matmul(
                        out,
                        w_tile(j, i),
                        src[:, B * j : B * (j + 1)],
                        start=False,
                        stop=(j == NCH - 1),
                    )
            dst = Hring[:, s * HB : (s + 1) * HB]
            nc.scalar.activation(dst[:, : 8 * B], ps_a[:], AF.Tanh, scale=1.0 / WSCALE)
            nc.scalar.activation(dst[:, 8 * B :], ps_b[:], AF.Tanh, scale=1.0 / WSCALE)

        for _rep in range(repeat):
            # h_{-1} = 0 lives in slot RING-1 (read by step 0).
            nc.vector.memset(Hring[:, (RING - 1) * HB : RING * HB], 0.0)
            with tc.For_i(
                0, T, RING, hint_engines=(mybir.EngineType.PE,)
            ) as iv:
                for s in range(RING):
                    step(iv + s, s)
                # Readout for the RING steps just produced:
                # y[o, (t, b)] = sum_k W_out[o, 128k+p] h[128k+p, (t, b)]
                yp = ypsum_pool.tile([N_OUT, RING * B], F32, tag="yp")
                for k in range(NCH):
                    nc.tensor.matmul(
                        yp[:],
                        wo_sb[:, N_OUT * k : N_OUT * (k + 1)],
                        Hr3[:, :, B * k : B * (k + 1)],
                        start=(k == 0),
                        stop=(k == NCH - 1),
                    )
                ysb = ysb_pool.tile([N_OUT, RING * B], F32, tag="ysb")
                nc.vector.tensor_scalar_add(ysb[:], yp[:], bo_sb[:, 0:1])
                nc.sync.dma_start(y[:, ds(iv * B, RING * B)], ysb[:])

    nc.finalize()
    return nc


def prep_inputs(x, W_in, W_res, b_res, W_out, b_out, T=T_FULL):
    bf = ml_dtypes.bfloat16
    f8 = ml_dtypes.float8_e4m3
    Wp = np.concatenate(
        [
            W_res,
            W_in,
            b_res[:, None],
            np.zeros((N_RES, KDIM - N_RES - N_IN - 1), np.float32),
        ],
        axis=1,
    )
    WpT = np.ascontiguousarray(Wp.T) * np.float32(WSCALE)  # [2176, 2048]
    wt8 = WpT[: NF8 * 128].astype(f8)
    wtb = WpT[NF8 * 128 :].astype(bf)
    wo = np.ascontiguousarray(W_out.T).astype(bf)         # [2048, 64]
    bo = np.ascontiguousarray(b_out[:, None]).astype(np.float32)
    xs = x[:, :T]                                         # [B, T, N_IN]
    xbc = np.zeros((128, T * B), bf)
    xbc[:N_IN] = (
        np.ascontiguousarray(xs.transpose(2, 1, 0).reshape(N_IN, T * B))
        .astype(bf)
    )
    xbc[N_IN] = bf(1.0)
    return [{"wt8": wt8, "wtb": wtb, "xb": xbc, "wo": wo, "bo": bo}]


def assemble_output(results, T=T_FULL):
    yc = results[0]["y"]                                  # [64, T*B]
    return np.ascontiguousarray(
        yc.reshape(N_OUT, T, B).transpose(2, 1, 0)
    )


def run(x, W_in, W_res, b_res, W_out, b_out, T=T_FULL, **run_kwargs):
    global LAST_RESULTS
    in_maps = prep_inputs(x, W_in, W_res, b_res, W_out, b_out, T=T)
    nc = build_module(T=T)
    res = run_bass_kernel_spmd(
        nc, in_maps, core_ids=list(range(N_CORES)), **run_kwargs
    )
    LAST_RESULTS = res
    return assemble_output(res.results, T=T)


def kernel(x, W_in, W_res, b_res, W_out, b_out):
    return run(
        np.asarray(x, np.float32),
        np.asarray(W_in, np.float32),
        np.asarray(W_res, np.float32),
        np.asarray(b_res, np.float32),
        np.asarray(W_out, np.float32),
        np.asarray(b_out, np.float32),
    )
